# revision 11
# baseline (speedup 1.0000x reference)
"""Trainium2 Bass kernel for nn_LohaModule (LoHa low-rank-adapted linear).

Computes  y = x @ (W0 + s*(A1@B1)*(A2@B2))^T + bias  for
x [4, 4096, 4096] f32, W0 [4096, 4096] f32, rank-32 factors.

Strategy: 8-way data parallel over the 16384 token rows (2048 rows/core).
Each core:
  - casts+transposes its x shard to bf16 X^T resident in SBUF (DMA xbar),
  - streams W0 once, building per-n-tile W^T bf16 panels via cast-DMA +
    DMA-transpose, merging the low-rank delta on PE (rank-32 matmuls) + DVE,
  - accumulates y tiles in PSUM over 32 k-tiles of bf16 matmuls, with the
    bias folded in via a K=1 ones-matmul,
  - stores fp32 output tiles; the host concatenates the 8 row shards.
"""

import os
import sys

for _p in ("/opt/trn_rl_repo",):
    if _p not in sys.path and os.path.isdir(_p):
        sys.path.append(_p)

from contextlib import ExitStack
from dataclasses import dataclass

import numpy as np

import concourse.bass as bass
import concourse.tile as tile
from concourse import bacc, mybir
from concourse.bass_utils import run_bass_kernel_spmd
from concourse.masks import make_identity

N_CORES = 8
BATCH, SEQ = 4, 4096
IN_DIM = 4096
OUT_DIM = 4096
RANK = 32
SCALE = 16.0 / 32.0  # alpha / rank * multiplier * scalar
P = 128

f32 = mybir.dt.float32
bf16 = mybir.dt.bfloat16
Copy = mybir.ActivationFunctionType.Copy


@dataclass(frozen=True)
class Cfg:
    tok: int  # token rows per core
    in_dim: int
    out_dim: int
    rank: int
    n_tile: int = 256  # output-feature tile (PSUM free dim)

    @property
    def kt(self):
        return self.in_dim // P

    @property
    def mt(self):
        return self.tok // P

    @property
    def nt(self):
        return self.out_dim // self.n_tile

    @property
    def oc(self):
        return self.n_tile // P

    @property
    def sw(self):
        return min(2048, self.in_dim)  # X stage width for cast+transpose

    @property
    def xh(self):
        return self.in_dim // self.sw

    @property
    def skt(self):
        return self.sw // P

    @property
    def wsw(self):
        return min(1024, self.in_dim)  # W stage width

    @property
    def wh(self):
        return self.in_dim // self.wsw

    @property
    def wskt(self):
        return self.wsw // P


FULL_CFG = Cfg(tok=BATCH * SEQ // N_CORES, in_dim=IN_DIM, out_dim=OUT_DIM, rank=RANK)


def emit(ctx: ExitStack, tc: tile.TileContext, aps: dict, cfg: Cfg):
    """Emit the per-core kernel IR. aps: name -> bass.AP for the DRAM tensors."""
    nc = tc.nc
    x, w0, bias_d = aps["x"], aps["org_weight"], aps["org_bias"]
    w1a, w1b = aps["hada_w1_a"], aps["hada_w1_b"]
    w2a, w2b = aps["hada_w2_a"], aps["hada_w2_b"]
    y = aps["y"]
    KT, MT, NT, OC = cfg.kt, cfg.mt, cfg.nt, cfg.oc
    SW, XH, SKT, NTILE = cfg.sw, cfg.xh, cfg.skt, cfg.n_tile

    # DRAM scratch for the transposed (and for A1, pre-scaled) low-rank factors
    a1t_d = nc.dram_tensor("a1t_scratch", [cfg.rank, cfg.out_dim], bf16, kind="Internal").ap()
    a2t_d = nc.dram_tensor("a2t_scratch", [cfg.rank, cfg.out_dim], bf16, kind="Internal").ap()

    xt_pool = ctx.enter_context(tc.tile_pool(name="xt", bufs=1))
    wp_pool = ctx.enter_context(tc.tile_pool(name="wp", bufs=2))
    xstage_pool = ctx.enter_context(tc.tile_pool(name="xstage", bufs=2))
    wstage_pool = ctx.enter_context(tc.tile_pool(name="wstage", bufs=4))
    fac_pool = ctx.enter_context(tc.tile_pool(name="fac", bufs=1))
    an_pool = ctx.enter_context(tc.tile_pool(name="an", bufs=2))
    prod_pool = ctx.enter_context(tc.tile_pool(name="prod", bufs=2))
    ost_pool = ctx.enter_context(tc.tile_pool(name="ost", bufs=3))
    apre_pool = ctx.enter_context(tc.tile_pool(name="apre", bufs=2))
    mmps = ctx.enter_context(tc.tile_pool(name="mmps", bufs=4, space="PSUM"))
    tpps = ctx.enter_context(tc.tile_pool(name="tpps", bufs=4, space="PSUM"))

    # ---- constants / small factors ----
    ident = fac_pool.tile([P, P], bf16, tag="ident")
    make_identity(nc, ident[:])
    ones = fac_pool.tile([1, P], bf16, tag="ones")
    nc.gpsimd.memset(ones[:], 1.0)
    b1c = fac_pool.tile([cfg.rank, cfg.in_dim], bf16, tag="b1")
    nc.gpsimd.dma_start(b1c[:], w1b[:])
    b2c = fac_pool.tile([cfg.rank, cfg.in_dim], bf16, tag="b2")
    nc.gpsimd.dma_start(b2c[:], w2b[:])

    # ---- A1^T (pre-scaled by SCALE) and A2^T to DRAM scratch ----
    for src, dst, scale in ((w1a, a1t_d, SCALE), (w2a, a2t_d, 1.0)):
        for j in range(cfg.out_dim // P):
            a32 = apre_pool.tile([P, cfg.rank], f32, tag="a32")
            nc.sync.dma_start(a32[:], src[j * P : (j + 1) * P, :])
            a16 = apre_pool.tile([P, cfg.rank], bf16, tag="a16")
            nc.scalar.activation(a16[:], a32[:], Copy, scale=float(scale))
            aps_t = mmps.tile([cfg.rank, P], bf16, tag="mm")
            nc.tensor.transpose(aps_t[:], a16[:], ident[:])
            a_sb = apre_pool.tile([cfg.rank, P], bf16, tag="asb")
            nc.vector.tensor_copy(a_sb[:], aps_t[:])
            nc.sync.dma_start(dst[:, j * P : (j + 1) * P], a_sb[:])

    # ---- X^T resident (bf16), via cast-DMA + xbar transpose ----
    # One tile per m-tile so the first matmuls only wait for their own slice.
    xts = []
    for mt in range(MT):
        xt_m = xt_pool.tile([P, KT, P], bf16, tag=f"xt{mt}")
        xts.append(xt_m)
        for h in range(XH):
            st = xstage_pool.tile([P, SW], bf16, tag="xstage")
            nc.gpsimd.dma_start(st[:], x[mt * P : (mt + 1) * P, h * SW : (h + 1) * SW])
            nc.sync.dma_start(
                xt_m[:, h * SKT : (h + 1) * SKT, :],
                st[:],
                transpose=True,
            )

    # ---- main loop over output-feature tiles ----
    for n in range(NT):
        pn = wp_pool.tile([P, KT, NTILE], bf16, tag="wp")
        # W0^T panel via cast-DMA + xbar
        for oc in range(OC):
            o0 = n * NTILE + oc * P
            for h in range(cfg.wh):
                st = wstage_pool.tile([P, cfg.wsw], bf16, tag="wstage")
                nc.gpsimd.dma_start(
                    st[:], w0[o0 : o0 + P, h * cfg.wsw : (h + 1) * cfg.wsw]
                )
                nc.sync.dma_start(
                    pn[:, h * cfg.wskt : (h + 1) * cfg.wskt, oc * P : (oc + 1) * P],
                    st[:],
                    transpose=True,
                )
        # low-rank delta: pn[:, k, :] += (B1k^T A1n) * (B2k^T A2n)  (SCALE in A1n)
        a1n = an_pool.tile([cfg.rank, NTILE], bf16, tag="a1n")
        nc.sync.dma_start(a1n[:], a1t_d[:, n * NTILE : (n + 1) * NTILE])
        a2n = an_pool.tile([cfg.rank, NTILE], bf16, tag="a2n")
        nc.sync.dma_start(a2n[:], a2t_d[:, n * NTILE : (n + 1) * NTILE])
        for k in range(KT):
            t1 = tpps.tile([P, NTILE], f32, tag="tp")
            nc.tensor.matmul(t1[:], b1c[:, k * P : (k + 1) * P], a1n[:], start=True, stop=True)
            t2 = tpps.tile([P, NTILE], f32, tag="tp")
            nc.tensor.matmul(t2[:], b2c[:, k * P : (k + 1) * P], a2n[:], start=True, stop=True)
            # HW allows only one PSUM input per DVE op: stage t1 in SBUF via ACT
            t1sb = prod_pool.tile([P, NTILE], bf16, tag="t1sb")
            nc.scalar.activation(t1sb[:], t1[:], Copy)
            pr = prod_pool.tile([P, NTILE], bf16, tag="prod")
            nc.vector.tensor_mul(pr[:], t2[:], t1sb[:])
            nc.gpsimd.tensor_add(pn[:, k, :], pr[:], pn[:, k, :])
        # consume: y[m, n] = X^T_m^T @ panel + bias
        bias_n = an_pool.tile([1, NTILE], bf16, tag="biasn")
        nc.gpsimd.dma_start(bias_n[:], bias_d[n * NTILE : (n + 1) * NTILE])
        for m in range(MT):
            ps = mmps.tile([P, NTILE], f32, tag="mm")
            nc.tensor.matmul(
                ps[:],
                ones[:],
                bias_n[:],
                start=True,
                stop=False,
            )
            for k in range(KT):
                nc.tensor.matmul(
                    ps[:],
                    xts[m][:, k, :],
                    pn[:, k, :],
                    start=False,
                    stop=(k == KT - 1),
                )
            ost = ost_pool.tile([P, NTILE], f32, tag="ost")
            nc.scalar.activation(ost[:], ps[:], Copy)
            # issue the store from ACT: it follows the ACTIVATE in the same
            # FIFO, so it never head-of-line-blocks the SP queue's xbars
            nc.scalar.dma_start(
                y[m * P : (m + 1) * P, n * NTILE : (n + 1) * NTILE], ost[:]
            )


def build_nc(cfg: Cfg):
    nc = bacc.Bacc("TRN2", target_bir_lowering=False, debug=False, num_devices=N_CORES)
    aps = {
        "x": nc.dram_tensor("x", [cfg.tok, cfg.in_dim], f32, kind="ExternalInput").ap(),
        "org_weight": nc.dram_tensor(
            "org_weight", [cfg.out_dim, cfg.in_dim], f32, kind="ExternalInput"
        ).ap(),
        "org_bias": nc.dram_tensor(
            "org_bias", [cfg.out_dim], f32, kind="ExternalInput"
        ).ap(),
        "hada_w1_a": nc.dram_tensor(
            "hada_w1_a", [cfg.out_dim, cfg.rank], f32, kind="ExternalInput"
        ).ap(),
        "hada_w1_b": nc.dram_tensor(
            "hada_w1_b", [cfg.rank, cfg.in_dim], f32, kind="ExternalInput"
        ).ap(),
        "hada_w2_a": nc.dram_tensor(
            "hada_w2_a", [cfg.out_dim, cfg.rank], f32, kind="ExternalInput"
        ).ap(),
        "hada_w2_b": nc.dram_tensor(
            "hada_w2_b", [cfg.rank, cfg.in_dim], f32, kind="ExternalInput"
        ).ap(),
        "y": nc.dram_tensor("y", [cfg.tok, cfg.out_dim], f32, kind="ExternalOutput").ap(),
    }
    with tile.TileContext(nc) as tc:
        with ExitStack() as ctx:
            emit(ctx, tc, aps, cfg)
    nc.compile()
    return nc


_NC_CACHE: dict = {}


def _get_nc(cfg: Cfg):
    if cfg not in _NC_CACHE:
        _NC_CACHE[cfg] = build_nc(cfg)
    return _NC_CACHE[cfg]


def kernel(x, org_weight, org_bias, hada_w1_a, hada_w1_b, hada_w2_a, hada_w2_b, **run_kwargs):
    cfg = FULL_CFG
    x = np.ascontiguousarray(np.asarray(x, dtype=np.float32)).reshape(-1, cfg.in_dim)
    w0 = np.ascontiguousarray(np.asarray(org_weight, dtype=np.float32))
    b = np.ascontiguousarray(np.asarray(org_bias, dtype=np.float32))
    a1 = np.ascontiguousarray(np.asarray(hada_w1_a, dtype=np.float32))
    b1 = np.ascontiguousarray(np.asarray(hada_w1_b, dtype=np.float32))
    a2 = np.ascontiguousarray(np.asarray(hada_w2_a, dtype=np.float32))
    b2 = np.ascontiguousarray(np.asarray(hada_w2_b, dtype=np.float32))

    nc = _get_nc(cfg)
    in_maps = []
    for c in range(N_CORES):
        shard = x[c * cfg.tok : (c + 1) * cfg.tok]
        in_maps.append(
            {
                "x": shard,
                "org_weight": w0,
                "org_bias": b,
                "hada_w1_a": a1,
                "hada_w1_b": b1,
                "hada_w2_a": a2,
                "hada_w2_b": b2,
            }
        )
    res = run_bass_kernel_spmd(nc, in_maps, core_ids=list(range(N_CORES)), **run_kwargs)
    out = np.concatenate([res.results[c]["y"] for c in range(N_CORES)], axis=0)
    out = out.reshape(BATCH, SEQ, cfg.out_dim)
    if run_kwargs:
        kernel.last_results = res  # for test harness introspection
    return out


if __name__ == "__main__":
    # quick shape sanity of the full build (no run)
    nc = build_nc(FULL_CFG)
    print("built OK:", len(nc.m.functions[0].instructions), "instructions")


# revision 15
# speedup vs baseline: 1.0959x; 1.0959x over previous
"""Trainium2 Bass kernel for nn_LohaModule (LoHa low-rank-adapted linear).

Computes  y = x @ (W0 + s*(A1@B1)*(A2@B2))^T + bias  for
x [4, 4096, 4096] f32, W0 [4096, 4096] f32, rank-32 factors.

Strategy: 8-way data parallel over the 16384 token rows (2048 rows/core).
Each core:
  - casts+transposes its x shard to bf16 X^T resident in SBUF (DMA xbar),
  - streams W0 once, building per-n-tile W^T bf16 panels via cast-DMA +
    DMA-transpose, merging the low-rank delta on PE (rank-32 matmuls) + DVE,
  - accumulates y tiles in PSUM over 32 k-tiles of bf16 matmuls, with the
    bias folded in via a K=1 ones-matmul,
  - stores fp32 output tiles; the host concatenates the 8 row shards.
"""

import os
import sys

for _p in ("/opt/trn_rl_repo",):
    if _p not in sys.path and os.path.isdir(_p):
        sys.path.append(_p)

from contextlib import ExitStack
from dataclasses import dataclass

import numpy as np

import concourse.bass as bass
import concourse.tile as tile
from concourse import bacc, mybir
from concourse.bass_utils import run_bass_kernel_spmd
from concourse.masks import make_identity

N_CORES = 8
BATCH, SEQ = 4, 4096
IN_DIM = 4096
OUT_DIM = 4096
RANK = 32
SCALE = 16.0 / 32.0  # alpha / rank * multiplier * scalar
P = 128

f32 = mybir.dt.float32
bf16 = mybir.dt.bfloat16
Copy = mybir.ActivationFunctionType.Copy


@dataclass(frozen=True)
class Cfg:
    tok: int  # token rows per core
    in_dim: int
    out_dim: int
    rank: int
    n_tile: int = 256  # output-feature tile (PSUM free dim)

    @property
    def kt(self):
        return self.in_dim // P

    @property
    def mt(self):
        return self.tok // P

    @property
    def nt(self):
        return self.out_dim // self.n_tile

    @property
    def oc(self):
        return self.n_tile // P

    @property
    def sw(self):
        return min(2048, self.in_dim)  # X stage width for cast+transpose

    @property
    def xh(self):
        return self.in_dim // self.sw

    @property
    def skt(self):
        return self.sw // P

    @property
    def wsw(self):
        return min(1024, self.in_dim)  # W stage width

    @property
    def wh(self):
        return self.in_dim // self.wsw

    @property
    def wskt(self):
        return self.wsw // P


FULL_CFG = Cfg(tok=BATCH * SEQ // N_CORES, in_dim=IN_DIM, out_dim=OUT_DIM, rank=RANK)


def emit(ctx: ExitStack, tc: tile.TileContext, aps: dict, cfg: Cfg):
    """Emit the per-core kernel IR. aps: name -> bass.AP for the DRAM tensors."""
    nc = tc.nc
    x, w0, bias_d = aps["x"], aps["org_weight"], aps["org_bias"]
    w1a, w1b = aps["hada_w1_a"], aps["hada_w1_b"]
    w2a, w2b = aps["hada_w2_a"], aps["hada_w2_b"]
    y = aps["y"]
    KT, MT, NT, OC = cfg.kt, cfg.mt, cfg.nt, cfg.oc
    SW, XH, SKT, NTILE = cfg.sw, cfg.xh, cfg.skt, cfg.n_tile

    # DRAM scratch for the transposed (and for A1, pre-scaled) low-rank factors
    a1t_d = nc.dram_tensor("a1t_scratch", [cfg.rank, cfg.out_dim], bf16, kind="Internal").ap()
    a2t_d = nc.dram_tensor("a2t_scratch", [cfg.rank, cfg.out_dim], bf16, kind="Internal").ap()

    xt_pool = ctx.enter_context(tc.tile_pool(name="xt", bufs=1))
    wp_pool = ctx.enter_context(tc.tile_pool(name="wp", bufs=2))
    xstage_pool = ctx.enter_context(tc.tile_pool(name="xstage", bufs=3))
    wstage_pool = ctx.enter_context(tc.tile_pool(name="wstage", bufs=4))
    fac_pool = ctx.enter_context(tc.tile_pool(name="fac", bufs=1))
    an_pool = ctx.enter_context(tc.tile_pool(name="an", bufs=3))
    prod_pool = ctx.enter_context(tc.tile_pool(name="prod", bufs=3))
    ost_pool = ctx.enter_context(tc.tile_pool(name="ost", bufs=2))
    apre_pool = ctx.enter_context(tc.tile_pool(name="apre", bufs=2))
    mmps = ctx.enter_context(tc.tile_pool(name="mmps", bufs=4, space="PSUM"))
    tpps = ctx.enter_context(tc.tile_pool(name="tpps", bufs=4, space="PSUM"))

    # ---- constants / small factors ----
    ident = fac_pool.tile([P, P], bf16, tag="ident")
    make_identity(nc, ident[:])
    ones = fac_pool.tile([1, P], bf16, tag="ones")
    nc.gpsimd.memset(ones[:], 1.0)
    b1c = fac_pool.tile([cfg.rank, cfg.in_dim], bf16, tag="b1")
    nc.gpsimd.dma_start(b1c[:], w1b[:])
    b2c = fac_pool.tile([cfg.rank, cfg.in_dim], bf16, tag="b2")
    nc.gpsimd.dma_start(b2c[:], w2b[:])

    # ---- A1^T (pre-scaled by SCALE) and A2^T to DRAM scratch ----
    for src, dst, scale in ((w1a, a1t_d, SCALE), (w2a, a2t_d, 1.0)):
        for j in range(cfg.out_dim // P):
            a32 = apre_pool.tile([P, cfg.rank], f32, tag="a32")
            nc.sync.dma_start(a32[:], src[j * P : (j + 1) * P, :])
            a16 = apre_pool.tile([P, cfg.rank], bf16, tag="a16")
            nc.scalar.activation(a16[:], a32[:], Copy, scale=float(scale))
            aps_t = mmps.tile([cfg.rank, P], bf16, tag="mm")
            nc.tensor.transpose(aps_t[:], a16[:], ident[:])
            a_sb = apre_pool.tile([cfg.rank, P], bf16, tag="asb")
            nc.vector.tensor_copy(a_sb[:], aps_t[:])
            nc.sync.dma_start(dst[:, j * P : (j + 1) * P], a_sb[:])

    # ---- panel build + consume, software-pipelined two panels ahead ----
    def build_panel(n):
        pn = wp_pool.tile([P, KT, NTILE], bf16, tag="wp")
        # W0^T panel via cast-DMA + xbar
        for oc in range(OC):
            o0 = n * NTILE + oc * P
            for h in range(cfg.wh):
                st = wstage_pool.tile([P, cfg.wsw], bf16, tag="wstage")
                nc.gpsimd.dma_start(
                    st[:], w0[o0 : o0 + P, h * cfg.wsw : (h + 1) * cfg.wsw]
                )
                nc.sync.dma_start(
                    pn[:, h * cfg.wskt : (h + 1) * cfg.wskt, oc * P : (oc + 1) * P],
                    st[:],
                    transpose=True,
                )
        # low-rank delta: pn[:, k, :] += (B1k^T A1n) * (B2k^T A2n)  (SCALE in A1n)
        a1n = an_pool.tile([cfg.rank, NTILE], bf16, tag="a1n")
        nc.sync.dma_start(a1n[:], a1t_d[:, n * NTILE : (n + 1) * NTILE])
        a2n = an_pool.tile([cfg.rank, NTILE], bf16, tag="a2n")
        nc.sync.dma_start(a2n[:], a2t_d[:, n * NTILE : (n + 1) * NTILE])
        for k in range(KT):
            t1 = tpps.tile([P, NTILE], f32, tag="tp")
            nc.tensor.matmul(t1[:], b1c[:, k * P : (k + 1) * P], a1n[:], start=True, stop=True)
            t2 = tpps.tile([P, NTILE], f32, tag="tp")
            nc.tensor.matmul(t2[:], b2c[:, k * P : (k + 1) * P], a2n[:], start=True, stop=True)
            # all-DVE merge chain (FIFO on one engine, no cross-engine hops);
            # HW allows only one PSUM input per DVE op, so stage t1 first
            t1sb = prod_pool.tile([P, NTILE], bf16, tag="t1sb")
            nc.vector.tensor_copy(t1sb[:], t1[:])
            pr = prod_pool.tile([P, NTILE], bf16, tag="prod")
            nc.vector.tensor_mul(pr[:], t2[:], t1sb[:])
            nc.vector.tensor_add(pn[:, k, :], pr[:], pn[:, k, :])
        bias_n = an_pool.tile([1, NTILE], bf16, tag="biasn")
        nc.gpsimd.dma_start(bias_n[:], bias_d[n * NTILE : (n + 1) * NTILE])
        return pn, bias_n

    def consume_panel(n, pn, bias_n):
        # y[m, n] = X^T_m^T @ panel + bias
        for m in range(MT):
            ps = mmps.tile([P, NTILE], f32, tag="mm")
            nc.tensor.matmul(ps[:], ones[:], bias_n[:], start=True, stop=False)
            for k in range(KT):
                nc.tensor.matmul(
                    ps[:],
                    xts[m][:, k, :],
                    pn[:, k, :],
                    start=False,
                    stop=(k == KT - 1),
                )
            ost = ost_pool.tile([P, NTILE], f32, tag="ost")
            nc.scalar.activation(ost[:], ps[:], Copy)
            # issue the store from ACT: it follows the ACTIVATE in the same
            # FIFO, so it never head-of-line-blocks the SP queue's xbars
            nc.scalar.dma_start(
                y[m * P : (m + 1) * P, n * NTILE : (n + 1) * NTILE], ost[:]
            )

    panels = {}
    panels[0] = build_panel(0)
    if NT > 1:
        panels[1] = build_panel(1)

    # ---- X^T resident (bf16), via cast-DMA + xbar transpose; emitted after
    # the first panel builds so those aren't queued behind 32 X chunks ----
    xts = []
    for mt in range(MT):
        xt_m = xt_pool.tile([P, KT, P], bf16, tag=f"xt{mt}")
        xts.append(xt_m)
        for h in range(XH):
            st = xstage_pool.tile([P, SW], bf16, tag="xstage")
            nc.gpsimd.dma_start(st[:], x[mt * P : (mt + 1) * P, h * SW : (h + 1) * SW])
            nc.sync.dma_start(
                xt_m[:, h * SKT : (h + 1) * SKT, :],
                st[:],
                transpose=True,
            )

    for n in range(NT):
        consume_panel(n, *panels.pop(n))
        if n + 2 < NT:
            panels[n + 2] = build_panel(n + 2)


def build_nc(cfg: Cfg):
    nc = bacc.Bacc("TRN2", target_bir_lowering=False, debug=False, num_devices=N_CORES)
    aps = {
        "x": nc.dram_tensor("x", [cfg.tok, cfg.in_dim], f32, kind="ExternalInput").ap(),
        "org_weight": nc.dram_tensor(
            "org_weight", [cfg.out_dim, cfg.in_dim], f32, kind="ExternalInput"
        ).ap(),
        "org_bias": nc.dram_tensor(
            "org_bias", [cfg.out_dim], f32, kind="ExternalInput"
        ).ap(),
        "hada_w1_a": nc.dram_tensor(
            "hada_w1_a", [cfg.out_dim, cfg.rank], f32, kind="ExternalInput"
        ).ap(),
        "hada_w1_b": nc.dram_tensor(
            "hada_w1_b", [cfg.rank, cfg.in_dim], f32, kind="ExternalInput"
        ).ap(),
        "hada_w2_a": nc.dram_tensor(
            "hada_w2_a", [cfg.out_dim, cfg.rank], f32, kind="ExternalInput"
        ).ap(),
        "hada_w2_b": nc.dram_tensor(
            "hada_w2_b", [cfg.rank, cfg.in_dim], f32, kind="ExternalInput"
        ).ap(),
        "y": nc.dram_tensor("y", [cfg.tok, cfg.out_dim], f32, kind="ExternalOutput").ap(),
    }
    with tile.TileContext(nc) as tc:
        with ExitStack() as ctx:
            emit(ctx, tc, aps, cfg)
    nc.compile()
    return nc


_NC_CACHE: dict = {}


def _get_nc(cfg: Cfg):
    if cfg not in _NC_CACHE:
        _NC_CACHE[cfg] = build_nc(cfg)
    return _NC_CACHE[cfg]


def kernel(x, org_weight, org_bias, hada_w1_a, hada_w1_b, hada_w2_a, hada_w2_b, **run_kwargs):
    cfg = FULL_CFG
    x = np.ascontiguousarray(np.asarray(x, dtype=np.float32)).reshape(-1, cfg.in_dim)
    w0 = np.ascontiguousarray(np.asarray(org_weight, dtype=np.float32))
    b = np.ascontiguousarray(np.asarray(org_bias, dtype=np.float32))
    a1 = np.ascontiguousarray(np.asarray(hada_w1_a, dtype=np.float32))
    b1 = np.ascontiguousarray(np.asarray(hada_w1_b, dtype=np.float32))
    a2 = np.ascontiguousarray(np.asarray(hada_w2_a, dtype=np.float32))
    b2 = np.ascontiguousarray(np.asarray(hada_w2_b, dtype=np.float32))

    nc = _get_nc(cfg)
    in_maps = []
    for c in range(N_CORES):
        shard = x[c * cfg.tok : (c + 1) * cfg.tok]
        in_maps.append(
            {
                "x": shard,
                "org_weight": w0,
                "org_bias": b,
                "hada_w1_a": a1,
                "hada_w1_b": b1,
                "hada_w2_a": a2,
                "hada_w2_b": b2,
            }
        )
    res = run_bass_kernel_spmd(nc, in_maps, core_ids=list(range(N_CORES)), **run_kwargs)
    out = np.concatenate([res.results[c]["y"] for c in range(N_CORES)], axis=0)
    out = out.reshape(BATCH, SEQ, cfg.out_dim)
    if run_kwargs:
        kernel.last_results = res  # for test harness introspection
    return out


if __name__ == "__main__":
    # quick shape sanity of the full build (no run)
    nc = build_nc(FULL_CFG)
    print("built OK:", len(nc.m.functions[0].instructions), "instructions")


# revision 18
# speedup vs baseline: 1.4210x; 1.2967x over previous
"""Trainium2 Bass kernel for nn_LohaModule (LoHa low-rank-adapted linear).

Computes  y = x @ (W0 + s*(A1@B1)*(A2@B2))^T + bias  for
x [4, 4096, 4096] f32, W0 [4096, 4096] f32, rank-32 factors.

Strategy: 8-way data parallel over the 16384 token rows (2048 rows/core).
Each core:
  - casts+transposes its x shard to bf16 X^T resident in SBUF (DMA xbar),
  - streams W0 once, building per-n-tile W^T bf16 panels via cast-DMA +
    DMA-transpose, merging the low-rank delta on PE (rank-32 matmuls) + DVE,
  - accumulates y tiles in PSUM over 32 k-tiles of bf16 matmuls, with the
    bias folded in via a K=1 ones-matmul,
  - stores fp32 output tiles; the host concatenates the 8 row shards.
"""

import os
import sys

for _p in ("/opt/trn_rl_repo",):
    if _p not in sys.path and os.path.isdir(_p):
        sys.path.append(_p)

from contextlib import ExitStack
from dataclasses import dataclass

import numpy as np

import concourse.bass as bass
import concourse.tile as tile
from concourse import bacc, mybir
from concourse.bass_utils import run_bass_kernel_spmd
from concourse.masks import make_identity

N_CORES = 8
BATCH, SEQ = 4, 4096
IN_DIM = 4096
OUT_DIM = 4096
RANK = 32
SCALE = 16.0 / 32.0  # alpha / rank * multiplier * scalar
P = 128

f32 = mybir.dt.float32
bf16 = mybir.dt.bfloat16
Copy = mybir.ActivationFunctionType.Copy


@dataclass(frozen=True)
class Cfg:
    tok: int  # token rows per core
    in_dim: int
    out_dim: int
    rank: int
    n_tile: int = 256  # output-feature tile (PSUM free dim)

    @property
    def kt(self):
        return self.in_dim // P

    @property
    def mt(self):
        return self.tok // P

    @property
    def nt(self):
        return self.out_dim // self.n_tile

    @property
    def oc(self):
        return self.n_tile // P

    @property
    def sw(self):
        return min(2048, self.in_dim)  # X stage width for cast+transpose

    @property
    def xh(self):
        return self.in_dim // self.sw

    @property
    def skt(self):
        return self.sw // P

    @property
    def wsw(self):
        return min(1024, self.in_dim)  # W stage width

    @property
    def wh(self):
        return self.in_dim // self.wsw

    @property
    def wskt(self):
        return self.wsw // P


FULL_CFG = Cfg(tok=BATCH * SEQ // N_CORES, in_dim=IN_DIM, out_dim=OUT_DIM, rank=RANK)


def emit(ctx: ExitStack, tc: tile.TileContext, aps: dict, cfg: Cfg):
    """Emit the per-core kernel IR. aps: name -> bass.AP for the DRAM tensors."""
    nc = tc.nc
    x, w0, bias_d = aps["x"], aps["org_weight"], aps["org_bias"]
    w1a, w1b = aps["hada_w1_a"], aps["hada_w1_b"]
    w2a, w2b = aps["hada_w2_a"], aps["hada_w2_b"]
    y = aps["y"]
    KT, MT, NT, OC = cfg.kt, cfg.mt, cfg.nt, cfg.oc
    SW, XH, SKT, NTILE = cfg.sw, cfg.xh, cfg.skt, cfg.n_tile

    # DRAM scratch for the transposed (and for A1, pre-scaled) low-rank factors
    a1t_d = nc.dram_tensor("a1t_scratch", [cfg.rank, cfg.out_dim], bf16, kind="Internal").ap()
    a2t_d = nc.dram_tensor("a2t_scratch", [cfg.rank, cfg.out_dim], bf16, kind="Internal").ap()

    xt_pool = ctx.enter_context(tc.tile_pool(name="xt", bufs=1))
    wp_pool = ctx.enter_context(tc.tile_pool(name="wp", bufs=3))
    fac_pool = ctx.enter_context(tc.tile_pool(name="fac", bufs=1))
    an_pool = ctx.enter_context(tc.tile_pool(name="an", bufs=4))
    prod_pool = ctx.enter_context(tc.tile_pool(name="prod", bufs=3))
    ost_pool = ctx.enter_context(tc.tile_pool(name="ost", bufs=2))
    apre_pool = ctx.enter_context(tc.tile_pool(name="apre", bufs=3))
    mmps = ctx.enter_context(tc.tile_pool(name="mmps", bufs=4, space="PSUM"))
    tpps = ctx.enter_context(tc.tile_pool(name="tpps", bufs=4, space="PSUM"))

    # ---- constants / small factors ----
    ident = fac_pool.tile([P, P], bf16, tag="ident")
    make_identity(nc, ident[:])
    ones = fac_pool.tile([1, P], bf16, tag="ones")
    nc.gpsimd.memset(ones[:], 1.0)
    b1c = fac_pool.tile([cfg.rank, cfg.in_dim], bf16, tag="b1")
    nc.gpsimd.dma_start(b1c[:], w1b[:])
    b2c = fac_pool.tile([cfg.rank, cfg.in_dim], bf16, tag="b2")
    nc.gpsimd.dma_start(b2c[:], w2b[:])

    # ---- A1^T (pre-scaled by SCALE) and A2^T to DRAM scratch ----
    for src, dst, scale in ((w1a, a1t_d, SCALE), (w2a, a2t_d, 1.0)):
        for j in range(cfg.out_dim // P):
            a32 = apre_pool.tile([P, cfg.rank], f32, tag="a32")
            nc.sync.dma_start(a32[:], src[j * P : (j + 1) * P, :])
            a16 = apre_pool.tile([P, cfg.rank], bf16, tag="a16")
            nc.scalar.activation(a16[:], a32[:], Copy, scale=float(scale))
            aps_t = mmps.tile([cfg.rank, P], bf16, tag="mm")
            nc.tensor.transpose(aps_t[:], a16[:], ident[:])
            a_sb = apre_pool.tile([cfg.rank, P], bf16, tag="asb")
            nc.vector.tensor_copy(a_sb[:], aps_t[:])
            nc.sync.dma_start(dst[:, j * P : (j + 1) * P], a_sb[:])

    # ---- DRAM bf16 staging (no SBUF staging tiles, no slot chains) ----
    w16 = [
        nc.dram_tensor(f"w16_{n}", [NTILE, cfg.in_dim], bf16, kind="Internal").ap()
        for n in range(NT)
    ]
    x16 = [
        nc.dram_tensor(f"x16_{mt}", [P, cfg.in_dim], bf16, kind="Internal").ap()
        for mt in range(MT)
    ]

    # Panel state: (pn, a1n, a2n, bias_n)
    def panel_dma(n):
        """Issue the DMAs for panel n: W0 cast to DRAM bf16, xbar into SBUF,
        and the small per-panel factor loads. The low-rank merge is emitted
        separately (interleaved into the previous panel's m-loop)."""
        pn = wp_pool.tile([P, KT, NTILE], bf16, tag="wp")
        nc.gpsimd.dma_start(w16[n][:], w0[n * NTILE : (n + 1) * NTILE, :])
        for oc in range(OC):
            nc.sync.dma_start(
                pn[:, :, oc * P : (oc + 1) * P],
                w16[n][oc * P : (oc + 1) * P, :],
                transpose=True,
            )
        a1n = an_pool.tile([cfg.rank, NTILE], bf16, tag="a1n")
        nc.sync.dma_start(a1n[:], a1t_d[:, n * NTILE : (n + 1) * NTILE])
        a2n = an_pool.tile([cfg.rank, NTILE], bf16, tag="a2n")
        nc.sync.dma_start(a2n[:], a2t_d[:, n * NTILE : (n + 1) * NTILE])
        bias_n = an_pool.tile([1, NTILE], bf16, tag="biasn")
        nc.gpsimd.dma_start(bias_n[:], bias_d[n * NTILE : (n + 1) * NTILE])
        return pn, a1n, a2n, bias_n

    def dw_pair(st, k):
        """pn[:, k, :] += (B1k^T A1n) * (B2k^T A2n)  (SCALE folded into A1n)."""
        pn, a1n, a2n, _ = st
        t1 = tpps.tile([P, NTILE], f32, tag="tp")
        nc.tensor.matmul(t1[:], b1c[:, k * P : (k + 1) * P], a1n[:], start=True, stop=True)
        t2 = tpps.tile([P, NTILE], f32, tag="tp")
        nc.tensor.matmul(t2[:], b2c[:, k * P : (k + 1) * P], a2n[:], start=True, stop=True)
        # all-DVE merge chain (FIFO on one engine, no cross-engine hops);
        # HW allows only one PSUM input per DVE op, so stage t1 first
        t1sb = prod_pool.tile([P, NTILE], bf16, tag="t1sb")
        nc.vector.tensor_copy(t1sb[:], t1[:])
        pr = prod_pool.tile([P, NTILE], bf16, tag="prod")
        nc.vector.tensor_mul(pr[:], t2[:], t1sb[:])
        nc.vector.tensor_add(pn[:, k, :], pr[:], pn[:, k, :])

    def consume_panel(n, st, st_next):
        """y[m, n] = X^T_m^T @ panel + bias; the next panel's low-rank merge
        is drip-fed between m-iterations so its matmuls never head-of-line
        block the PE queue and DVE keeps pace."""
        pn, _, _, bias_n = st
        per_iter = (KT + MT - 1) // MT if st_next is not None else 0
        ks = iter(range(KT))
        for m in range(MT):
            ps = mmps.tile([P, NTILE], f32, tag="mm")
            nc.tensor.matmul(ps[:], ones[:], bias_n[:], start=True, stop=False)
            for k in range(KT):
                nc.tensor.matmul(
                    ps[:],
                    xts[m][:, k, :],
                    pn[:, k, :],
                    start=False,
                    stop=(k == KT - 1),
                )
            ost = ost_pool.tile([P, NTILE], f32, tag="ost")
            nc.scalar.activation(ost[:], ps[:], Copy)
            # issue the store from ACT: it follows the ACTIVATE in the same
            # FIFO, so it never head-of-line-blocks the SP queue's xbars
            nc.scalar.dma_start(
                y[m * P : (m + 1) * P, n * NTILE : (n + 1) * NTILE], ost[:]
            )
            for _ in range(per_iter):
                k = next(ks, None)
                if k is not None:
                    dw_pair(st_next, k)

    panels = {}
    panels[0] = panel_dma(0)
    if NT > 1:
        panels[1] = panel_dma(1)

    # ---- X^T resident (bf16): cast to DRAM bf16, then xbar straight into
    # SBUF; all casts are independent so the SDMA pool runs them flat out ----
    xts = []
    for mt in range(MT):
        nc.gpsimd.dma_start(x16[mt][:], x[mt * P : (mt + 1) * P, :])
        xt_m = xt_pool.tile([P, KT, P], bf16, tag=f"xt{mt}")
        xts.append(xt_m)
        nc.sync.dma_start(xt_m[:], x16[mt][:], transpose=True)

    # panel 0's merge runs up front (overlaps the X prep trickle)
    for k in range(KT):
        dw_pair(panels[0], k)

    for n in range(NT):
        if n + 2 < NT:
            panels[n + 2] = panel_dma(n + 2)
        consume_panel(n, panels.pop(n), panels.get(n + 1))


def build_nc(cfg: Cfg):
    nc = bacc.Bacc("TRN2", target_bir_lowering=False, debug=False, num_devices=N_CORES)
    aps = {
        "x": nc.dram_tensor("x", [cfg.tok, cfg.in_dim], f32, kind="ExternalInput").ap(),
        "org_weight": nc.dram_tensor(
            "org_weight", [cfg.out_dim, cfg.in_dim], f32, kind="ExternalInput"
        ).ap(),
        "org_bias": nc.dram_tensor(
            "org_bias", [cfg.out_dim], f32, kind="ExternalInput"
        ).ap(),
        "hada_w1_a": nc.dram_tensor(
            "hada_w1_a", [cfg.out_dim, cfg.rank], f32, kind="ExternalInput"
        ).ap(),
        "hada_w1_b": nc.dram_tensor(
            "hada_w1_b", [cfg.rank, cfg.in_dim], f32, kind="ExternalInput"
        ).ap(),
        "hada_w2_a": nc.dram_tensor(
            "hada_w2_a", [cfg.out_dim, cfg.rank], f32, kind="ExternalInput"
        ).ap(),
        "hada_w2_b": nc.dram_tensor(
            "hada_w2_b", [cfg.rank, cfg.in_dim], f32, kind="ExternalInput"
        ).ap(),
        "y": nc.dram_tensor("y", [cfg.tok, cfg.out_dim], f32, kind="ExternalOutput").ap(),
    }
    with tile.TileContext(nc) as tc:
        with ExitStack() as ctx:
            emit(ctx, tc, aps, cfg)
    nc.compile()
    return nc


_NC_CACHE: dict = {}


def _get_nc(cfg: Cfg):
    if cfg not in _NC_CACHE:
        _NC_CACHE[cfg] = build_nc(cfg)
    return _NC_CACHE[cfg]


def kernel(x, org_weight, org_bias, hada_w1_a, hada_w1_b, hada_w2_a, hada_w2_b, **run_kwargs):
    cfg = FULL_CFG
    x = np.ascontiguousarray(np.asarray(x, dtype=np.float32)).reshape(-1, cfg.in_dim)
    w0 = np.ascontiguousarray(np.asarray(org_weight, dtype=np.float32))
    b = np.ascontiguousarray(np.asarray(org_bias, dtype=np.float32))
    a1 = np.ascontiguousarray(np.asarray(hada_w1_a, dtype=np.float32))
    b1 = np.ascontiguousarray(np.asarray(hada_w1_b, dtype=np.float32))
    a2 = np.ascontiguousarray(np.asarray(hada_w2_a, dtype=np.float32))
    b2 = np.ascontiguousarray(np.asarray(hada_w2_b, dtype=np.float32))

    nc = _get_nc(cfg)
    in_maps = []
    for c in range(N_CORES):
        shard = x[c * cfg.tok : (c + 1) * cfg.tok]
        in_maps.append(
            {
                "x": shard,
                "org_weight": w0,
                "org_bias": b,
                "hada_w1_a": a1,
                "hada_w1_b": b1,
                "hada_w2_a": a2,
                "hada_w2_b": b2,
            }
        )
    res = run_bass_kernel_spmd(nc, in_maps, core_ids=list(range(N_CORES)), **run_kwargs)
    out = np.concatenate([res.results[c]["y"] for c in range(N_CORES)], axis=0)
    out = out.reshape(BATCH, SEQ, cfg.out_dim)
    if run_kwargs:
        kernel.last_results = res  # for test harness introspection
    return out


if __name__ == "__main__":
    # quick shape sanity of the full build (no run)
    nc = build_nc(FULL_CFG)
    print("built OK:", len(nc.m.functions[0].instructions), "instructions")
